# revision 41
# baseline (speedup 1.0000x reference)
"""Trainium2 Bass kernel for ExponentialSmoothing (EMA over time).

Reference: y[b, 0] = x[b, 0]; y[b, t] = alpha*x[b, t] + (1-alpha)*y[b, t-1],
x: [8, 8192, 512] fp32, alpha = 0.1.

Strategy
--------
Data-parallel over batch: core i processes x[i] ([8192, 512]).

Within a core, the EMA along T is computed as a blocked causal convolution
on the TensorEngine. Because (1-alpha)^k decays geometrically, y at time
t = 128*k + i only depends (above fp32 precision) on inputs with lag
<= i + 128: the truncation error of a two-block window is
alpha*(0.9^129)/sqrt(1-0.81) ~ 3e-7 absolute (y std ~0.23), i.e. ~1e-6
relative. So for each output block of 128 timesteps:

    y_blk[k] = Wp.T @ x_blk[k-1] + Wc.T @ x_blk[k]   (PSUM accumulate)

with Wc[j, i] = alpha*0.9^(i-j) (i >= j), Wp[j, i] = alpha*0.9^(i+128-j).
Blocks 0 and 1 use exact special-cased weights for the x[0] column
(y_0 = x_0 exactly).

fp32 matmuls run at 4 cyc/row on the PE and made the first version
PE-bound (136 us vs the ~90 us HBM roofline). Instead the host splits
every operand into an fp16 hi/lo pair (xh = fp16(x), xl = fp16(x - xh);
same for W), and each logical fp32 matmul becomes three 1-cyc/row fp16
matmuls accumulated in fp32 PSUM:

    W @ x ~= Wh@xh + Wl@xh + Wh@xl      (dropped Wl@xl ~ 2^-22 relative)

Input DMA bytes are unchanged (2 x fp16 = 4 B/elem), so the kernel sits
right at the HBM roofline (~34 MB/core at ~380 GB/s measured) with the
PE just underneath it (~88 us dense).

Measured engine/overhead layout that drove the remaining choices:
- input streams split across both HWDGE rings (xh on SyncE, xl on
  ScalarE), outputs on SWDGE (GpSimd) so neither ring head-of-line
  blocks; the last two small output chunks go back to HWDGE so the
  SWDGE queue drains before the kernel tail.
- all PSUM->SBUF copies on the Vector engine (ScalarE activates would
  pull in an ACT table load, and DVE has the headroom).
- chunk sizes ramp 1->8 blocks at the start (PE starts ~7.8 us in) and
  shrink at the end (short tail), with ~4 us of zero matmuls up front so
  the PE HAM clock gate is already open when real work lands.
"""

import numpy as np

import concourse.mybir as mybir
import concourse.tile as tile
from concourse import bacc
from concourse.bass_utils import run_bass_kernel_spmd
from concourse.vector_clock import ScopedClock


def _lean_drain_and_barrier(self, tick_clock, wait_clock):
    """TileContext._drain_and_barrier without the trailing all-engine
    barrier: engines halt at NEFF end anyway and every execution's preamble
    re-clears the semaphores, so the final barrier only adds ~2-4 us of
    kernel tail."""
    drain_inst = self.nc.sync.drain()
    wait_clock.add_sem_waits(
        drain_inst.ins, ScopedClock({None: tick_clock.global_clock})
    )
    self.nc.all_engine_barrier()
    assert self.sems is not None
    popped = self.nc._tile_sem_poison_stack.pop()
    assert popped is self._sem_poison
    self.nc.clear_and_free_semaphores(list(self.sems.allocated().values()))


tile.TileContext._drain_and_barrier = _lean_drain_and_barrier

ALPHA = 0.1
BETA = 1.0 - ALPHA
B, T, F = 8, 8192, 512
TB = 128                       # timesteps per block (= matmul M = PSUM partitions)
NBLK = T // TB                 # 64
N_CORES = 8

# test.py can flip these to get a profiled run
TRACE = False
TRACE_CORES = None
REPS = 1
LAST_EXEC_NS = None
LAST_ALL_NS = None
LAST_RESULTS = None

_cached_nc = None
_cached_weights = None


def _hi_lo(w):
    hi = w.astype(np.float16)
    lo = (w.astype(np.float64) - hi.astype(np.float64)).astype(np.float16)
    return np.ascontiguousarray(hi), np.ascontiguousarray(lo)


def _build_weights():
    """lhsT layout [t_in=j (partitions), t_out=i (free)]: entry = coeff of x_j in y_i."""
    i = np.arange(TB)[None, :].astype(np.float64)   # t_out
    j = np.arange(TB)[:, None].astype(np.float64)   # t_in
    wc = np.where(i >= j, ALPHA * BETA ** (i - j), 0.0)
    w0 = wc.copy()
    w0[0, :] = BETA ** i[0]                          # coeff of x_0 in y_i is 0.9^i
    wp = ALPHA * BETA ** (i + TB - j)
    wp1 = wp.copy()
    wp1[0, :] = BETA ** (i[0] + TB)
    out = {}
    for nm, w in (("w0", w0), ("wp1", wp1), ("wc", wc), ("wp", wp)):
        hi, lo = _hi_lo(w)
        out[nm + "h"] = hi
        out[nm + "l"] = lo
    # pack in W_NAMES order along the free dim: [128, 8*128]
    return np.ascontiguousarray(
        np.concatenate([out[nm] for nm in W_NAMES], axis=1)
    )


W_NAMES = ["w0h", "w0l", "wp1h", "wp1l", "wch", "wcl", "wph", "wpl"]

# chunk schedule: small chunks at both ends (fast PE start, short tail),
# 8-block (1 MiB fp16 in-DMA) steady state
CHUNK_SCHED = [1, 1, 2, 4] + [8] * 6 + [4, 2, 2]


def _build_program():
    assert sum(CHUNK_SCHED) == NBLK
    nc = bacc.Bacc(None)
    xh = nc.dram_tensor("xh", [T, F], mybir.dt.float16, kind="ExternalInput")
    xl = nc.dram_tensor("xl", [T, F], mybir.dt.float16, kind="ExternalInput")
    # all 8 [128, 128] weight matrices packed along the free dim -> one DMA
    wpack = nc.dram_tensor(
        "wpack", [TB, len(W_NAMES) * TB], mybir.dt.float16, kind="ExternalInput"
    )
    y = nc.dram_tensor("y", [T, F], mybir.dt.float32, kind="ExternalOutput")

    xhb = xh.rearrange("(k p) f -> p k f", p=TB)
    xlb = xl.rearrange("(k p) f -> p k f", p=TB)
    yb = y.rearrange("(k p) f -> p k f", p=TB)

    with tile.TileContext(nc) as tc:
        with (
            tc.tile_pool(name="consts", bufs=1) as cpool,
            tc.tile_pool(name="xin", bufs=7) as xpool,
            tc.tile_pool(name="yout", bufs=4) as ypool,
            tc.tile_pool(name="ps", bufs=8, space="PSUM") as pspool,
        ):
            wpk = cpool.tile([TB, len(W_NAMES) * TB], mybir.dt.float16, tag="wpack")
            nc.scalar.dma_start(out=wpk[:], in_=wpack[:])
            wt = {
                nm: wpk[:, wi * TB:(wi + 1) * TB]
                for wi, nm in enumerate(W_NAMES)
            }

            # PE warm-up: ~4 us of dummy matmuls on a zeroed scratch tile so
            # the HAM clock gate opens (1.2 -> 2.4 GHz) while the first input
            # chunk is still in flight.
            warm = cpool.tile([TB, F], mybir.dt.float16, tag="warm")
            nc.gpsimd.memset(warm[:], 0.0)
            wps = pspool.tile([TB, F], mybir.dt.float32, tag="ps")
            for wi in range(20):
                nc.tensor.matmul(
                    wps[:], warm[:, :TB], warm[:], start=(wi == 0), stop=(wi == 19)
                )

            prev_h = prev_l = None
            k0 = 0
            for c, nblk in enumerate(CHUNK_SCHED):
                xht = xpool.tile([TB, nblk * F], mybir.dt.float16, tag="xh")
                xlt = xpool.tile([TB, nblk * F], mybir.dt.float16, tag="xl")
                ihalves = 8 if nblk >= 8 else (2 if nblk >= 4 else 1)
                iper = nblk // ihalves
                for hh in range(ihalves):
                    s0, s1 = hh * iper, (hh + 1) * iper
                    nc.sync.dma_start(
                        out=xht[:, s0 * F:s1 * F].rearrange(
                            "p (n f) -> p n f", n=iper
                        ),
                        in_=xhb[:, k0 + s0:k0 + s1],
                    )
                    nc.scalar.dma_start(
                        out=xlt[:, s0 * F:s1 * F].rearrange(
                            "p (n f) -> p n f", n=iper
                        ),
                        in_=xlb[:, k0 + s0:k0 + s1],
                    )
                yt = ypool.tile([TB, nblk * F], mybir.dt.float32)
                for b in range(nblk):
                    k = k0 + b
                    ps = pspool.tile([TB, F], mybir.dt.float32)
                    cur_h = xht[:, b * F:(b + 1) * F]
                    cur_l = xlt[:, b * F:(b + 1) * F]
                    if k == 0:
                        mms = [
                            (wt["w0h"], cur_h),
                            (wt["w0l"], cur_h),
                            (wt["w0h"], cur_l),
                        ]
                    else:
                        if b > 0:
                            pv_h = xht[:, (b - 1) * F:b * F]
                            pv_l = xlt[:, (b - 1) * F:b * F]
                        else:
                            pv_h = prev_h[:, -F:]
                            pv_l = prev_l[:, -F:]
                        wph = wt["wp1h"] if k == 1 else wt["wph"]
                        wpl = wt["wp1l"] if k == 1 else wt["wpl"]
                        mms = [
                            (wph, pv_h),
                            (wpl, pv_h),
                            (wt["wch"], cur_h),
                            (wt["wcl"], cur_h),
                            (wph, pv_l),
                            (wt["wch"], cur_l),
                        ]
                    for mi, (lhsT, rhs) in enumerate(mms):
                        nc.tensor.matmul(
                            ps[:],
                            lhsT,
                            rhs,
                            start=(mi == 0),
                            stop=(mi == len(mms) - 1),
                        )
                    dst = yt[:, b * F:(b + 1) * F]
                    nc.vector.tensor_copy(dst, ps[:])
                # last small chunks go out via the HWDGE rings so the SWDGE
                # queue drains early (its kernel-tail drain is ~5 us when hot)
                out_eng = (
                    nc.gpsimd
                    if c < len(CHUNK_SCHED) - 2
                    else (nc.sync if c % 2 == 0 else nc.scalar)
                )
                halves = 4 if nblk >= 8 else 1
                per = nblk // halves
                for hh in range(halves):
                    out_eng.dma_start(
                        out=yb[:, k0 + hh * per:k0 + (hh + 1) * per],
                        in_=yt[:, hh * per * F:(hh + 1) * per * F].rearrange(
                            "p (n f) -> p n f", n=per
                        ),
                    )
                prev_h, prev_l = xht, xlt
                k0 += nblk
    nc.finalize()
    return nc


def kernel(**inputs) -> np.ndarray:
    global _cached_nc, _cached_weights, LAST_EXEC_NS, LAST_ALL_NS, LAST_RESULTS
    x = np.asarray(inputs["x"], dtype=np.float32)
    assert x.shape == (B, T, F), x.shape

    if _cached_weights is None:
        _cached_weights = _build_weights()
    if _cached_nc is None:
        _cached_nc = _build_program()

    xh = x.astype(np.float16)
    xl = (x.astype(np.float64) - xh.astype(np.float64)).astype(np.float16)

    in_maps = [
        {
            "xh": np.ascontiguousarray(xh[i]),
            "xl": np.ascontiguousarray(xl[i]),
            "wpack": _cached_weights,
        }
        for i in range(N_CORES)
    ]
    times = []
    for _ in range(max(1, REPS)):
        res = run_bass_kernel_spmd(
            _cached_nc,
            in_maps,
            core_ids=list(range(N_CORES)),
            trace=TRACE,
            trace_cores=TRACE_CORES,
        )
        if res.exec_time_ns is not None:
            times.append(res.exec_time_ns)
    LAST_ALL_NS = times
    LAST_EXEC_NS = min(times) if times else None
    LAST_RESULTS = res
    return np.stack([r["y"] for r in res.results], axis=0)


# revision 42
# speedup vs baseline: 1.0463x; 1.0463x over previous
"""Trainium2 Bass kernel for ExponentialSmoothing (EMA over time).

Reference: y[b, 0] = x[b, 0]; y[b, t] = alpha*x[b, t] + (1-alpha)*y[b, t-1],
x: [8, 8192, 512] fp32, alpha = 0.1.

Strategy
--------
Data-parallel over batch: core i processes x[i] ([8192, 512]).

Within a core, the EMA along T is computed as a blocked causal convolution
on the TensorEngine. Because (1-alpha)^k decays geometrically, y at time
t = 128*k + i only depends (above fp32 precision) on inputs with lag
<= i + 128: the truncation error of a two-block window is
alpha*(0.9^129)/sqrt(1-0.81) ~ 3e-7 absolute (y std ~0.23), i.e. ~1e-6
relative. So for each output block of 128 timesteps:

    y_blk[k] = Wp.T @ x_blk[k-1] + Wc.T @ x_blk[k]   (PSUM accumulate)

with Wc[j, i] = alpha*0.9^(i-j) (i >= j), Wp[j, i] = alpha*0.9^(i+128-j).
Blocks 0 and 1 use exact special-cased weights for the x[0] column
(y_0 = x_0 exactly).

fp32 matmuls run at 4 cyc/row on the PE and made the first version
PE-bound (136 us vs the ~90 us HBM roofline). Instead the host splits
every operand into an fp16 hi/lo pair (xh = fp16(x), xl = fp16(x - xh);
same for W), and each logical fp32 matmul becomes three 1-cyc/row fp16
matmuls accumulated in fp32 PSUM:

    W @ x ~= Wh@xh + Wl@xh + Wh@xl      (dropped Wl@xl ~ 2^-22 relative)

Input DMA bytes are unchanged (2 x fp16 = 4 B/elem), so the kernel sits
right at the HBM roofline (~34 MB/core at ~380 GB/s measured) with the
PE just underneath it (~88 us dense).

Measured engine/overhead layout that drove the remaining choices:
- input streams split across both HWDGE rings (xh on SyncE, xl on
  ScalarE), outputs on SWDGE (GpSimd) so neither ring head-of-line
  blocks; the last two small output chunks go back to HWDGE so the
  SWDGE queue drains before the kernel tail.
- all PSUM->SBUF copies on the Vector engine (ScalarE activates would
  pull in an ACT table load, and DVE has the headroom).
- chunk sizes ramp 1->8 blocks at the start (PE starts ~7.8 us in) and
  shrink at the end (short tail), with ~4 us of zero matmuls up front so
  the PE HAM clock gate is already open when real work lands.
"""

import numpy as np

import concourse.mybir as mybir
import concourse.tile as tile
from concourse import bacc
from concourse.bass_utils import run_bass_kernel_spmd
from concourse.vector_clock import ScopedClock


def _lean_drain_and_barrier(self, tick_clock, wait_clock):
    """TileContext._drain_and_barrier without the trailing all-engine
    barrier: engines halt at NEFF end anyway and every execution's preamble
    re-clears the semaphores, so the final barrier only adds ~2-4 us of
    kernel tail."""
    drain_inst = self.nc.sync.drain()
    wait_clock.add_sem_waits(
        drain_inst.ins, ScopedClock({None: tick_clock.global_clock})
    )
    self.nc.all_engine_barrier()
    assert self.sems is not None
    popped = self.nc._tile_sem_poison_stack.pop()
    assert popped is self._sem_poison
    self.nc.clear_and_free_semaphores(list(self.sems.allocated().values()))


tile.TileContext._drain_and_barrier = _lean_drain_and_barrier

ALPHA = 0.1
BETA = 1.0 - ALPHA
B, T, F = 8, 8192, 512
TB = 128                       # timesteps per block (= matmul M = PSUM partitions)
NBLK = T // TB                 # 64
N_CORES = 8

# test.py can flip these to get a profiled run
TRACE = False
TRACE_CORES = None
REPS = 1
LAST_EXEC_NS = None
LAST_ALL_NS = None
LAST_RESULTS = None

_cached_nc = None
_cached_weights = None


def _hi_lo(w):
    hi = w.astype(np.float16)
    lo = (w.astype(np.float64) - hi.astype(np.float64)).astype(np.float16)
    return np.ascontiguousarray(hi), np.ascontiguousarray(lo)


def _build_weights():
    """lhsT layout [t_in=j (partitions), t_out=i (free)]: entry = coeff of x_j in y_i."""
    i = np.arange(TB)[None, :].astype(np.float64)   # t_out
    j = np.arange(TB)[:, None].astype(np.float64)   # t_in
    wc = np.where(i >= j, ALPHA * BETA ** (i - j), 0.0)
    w0 = wc.copy()
    w0[0, :] = BETA ** i[0]                          # coeff of x_0 in y_i is 0.9^i
    wp = ALPHA * BETA ** (i + TB - j)
    wp1 = wp.copy()
    wp1[0, :] = BETA ** (i[0] + TB)
    out = {}
    for nm, w in (("w0", w0), ("wp1", wp1), ("wc", wc), ("wp", wp)):
        hi, lo = _hi_lo(w)
        out[nm + "h"] = hi
        out[nm + "l"] = lo
    # pack in W_NAMES order along the free dim: [128, 8*128]
    return np.ascontiguousarray(
        np.concatenate([out[nm] for nm in W_NAMES], axis=1)
    )


W_NAMES = ["w0h", "w0l", "wp1h", "wp1l", "wch", "wcl", "wph", "wpl"]

# chunk schedule: small chunks at both ends (fast PE start, short tail),
# 8-block (1 MiB fp16 in-DMA) steady state
CHUNK_SCHED = [1, 1, 2, 4] + [8] * 6 + [4, 2, 2]


def _build_program():
    assert sum(CHUNK_SCHED) == NBLK
    nc = bacc.Bacc(None)
    xh = nc.dram_tensor("xh", [T, F], mybir.dt.float16, kind="ExternalInput")
    xl = nc.dram_tensor("xl", [T, F], mybir.dt.float16, kind="ExternalInput")
    # all 8 [128, 128] weight matrices packed along the free dim -> one DMA
    wpack = nc.dram_tensor(
        "wpack", [TB, len(W_NAMES) * TB], mybir.dt.float16, kind="ExternalInput"
    )
    y = nc.dram_tensor("y", [T, F], mybir.dt.float32, kind="ExternalOutput")

    xhb = xh.rearrange("(k p) f -> p k f", p=TB)
    xlb = xl.rearrange("(k p) f -> p k f", p=TB)
    yb = y.rearrange("(k p) f -> p k f", p=TB)

    with tile.TileContext(nc) as tc:
        with (
            tc.tile_pool(name="consts", bufs=1) as cpool,
            tc.tile_pool(name="xin", bufs=7) as xpool,
            tc.tile_pool(name="yout", bufs=4) as ypool,
            tc.tile_pool(name="ps", bufs=8, space="PSUM") as pspool,
        ):
            wpk = cpool.tile([TB, len(W_NAMES) * TB], mybir.dt.float16, tag="wpack")
            nc.scalar.dma_start(out=wpk[:], in_=wpack[:])
            wt = {
                nm: wpk[:, wi * TB:(wi + 1) * TB]
                for wi, nm in enumerate(W_NAMES)
            }

            # PE warm-up: ~4 us of dummy matmuls on a zeroed scratch tile so
            # the HAM clock gate opens (1.2 -> 2.4 GHz) while the first input
            # chunk is still in flight.
            warm = cpool.tile([TB, F], mybir.dt.float16, tag="warm")
            nc.gpsimd.memset(warm[:], 0.0)
            wps = pspool.tile([TB, F], mybir.dt.float32, tag="ps")
            for wi in range(20):
                nc.tensor.matmul(
                    wps[:], warm[:, :TB], warm[:], start=(wi == 0), stop=(wi == 19)
                )

            prev_h = prev_l = None
            k0 = 0
            for c, nblk in enumerate(CHUNK_SCHED):
                xht = xpool.tile([TB, nblk * F], mybir.dt.float16, tag="xh")
                xlt = xpool.tile([TB, nblk * F], mybir.dt.float16, tag="xl")
                ihalves = 4 if nblk >= 8 else (2 if nblk >= 4 else 1)
                iper = nblk // ihalves
                for hh in range(ihalves):
                    s0, s1 = hh * iper, (hh + 1) * iper
                    nc.sync.dma_start(
                        out=xht[:, s0 * F:s1 * F].rearrange(
                            "p (n f) -> p n f", n=iper
                        ),
                        in_=xhb[:, k0 + s0:k0 + s1],
                    )
                    nc.scalar.dma_start(
                        out=xlt[:, s0 * F:s1 * F].rearrange(
                            "p (n f) -> p n f", n=iper
                        ),
                        in_=xlb[:, k0 + s0:k0 + s1],
                    )
                yt = ypool.tile([TB, nblk * F], mybir.dt.float32)
                for b in range(nblk):
                    k = k0 + b
                    ps = pspool.tile([TB, F], mybir.dt.float32)
                    cur_h = xht[:, b * F:(b + 1) * F]
                    cur_l = xlt[:, b * F:(b + 1) * F]
                    if k == 0:
                        mms = [
                            (wt["w0h"], cur_h),
                            (wt["w0l"], cur_h),
                            (wt["w0h"], cur_l),
                        ]
                    else:
                        if b > 0:
                            pv_h = xht[:, (b - 1) * F:b * F]
                            pv_l = xlt[:, (b - 1) * F:b * F]
                        else:
                            pv_h = prev_h[:, -F:]
                            pv_l = prev_l[:, -F:]
                        wph = wt["wp1h"] if k == 1 else wt["wph"]
                        wpl = wt["wp1l"] if k == 1 else wt["wpl"]
                        mms = [
                            (wph, pv_h),
                            (wpl, pv_h),
                            (wt["wch"], cur_h),
                            (wt["wcl"], cur_h),
                            (wph, pv_l),
                            (wt["wch"], cur_l),
                        ]
                    for mi, (lhsT, rhs) in enumerate(mms):
                        nc.tensor.matmul(
                            ps[:],
                            lhsT,
                            rhs,
                            start=(mi == 0),
                            stop=(mi == len(mms) - 1),
                        )
                    dst = yt[:, b * F:(b + 1) * F]
                    nc.vector.tensor_copy(dst, ps[:])
                # last small chunks go out via the HWDGE rings so the SWDGE
                # queue drains early (its kernel-tail drain is ~5 us when hot)
                out_eng = (
                    nc.gpsimd
                    if c < len(CHUNK_SCHED) - 2
                    else (nc.sync if c % 2 == 0 else nc.scalar)
                )
                halves = 2 if nblk >= 8 else 1
                per = nblk // halves
                for hh in range(halves):
                    out_eng.dma_start(
                        out=yb[:, k0 + hh * per:k0 + (hh + 1) * per],
                        in_=yt[:, hh * per * F:(hh + 1) * per * F].rearrange(
                            "p (n f) -> p n f", n=per
                        ),
                    )
                prev_h, prev_l = xht, xlt
                k0 += nblk
    nc.finalize()
    return nc


def kernel(**inputs) -> np.ndarray:
    global _cached_nc, _cached_weights, LAST_EXEC_NS, LAST_ALL_NS, LAST_RESULTS
    x = np.asarray(inputs["x"], dtype=np.float32)
    assert x.shape == (B, T, F), x.shape

    if _cached_weights is None:
        _cached_weights = _build_weights()
    if _cached_nc is None:
        _cached_nc = _build_program()

    xh = x.astype(np.float16)
    xl = (x.astype(np.float64) - xh.astype(np.float64)).astype(np.float16)

    in_maps = [
        {
            "xh": np.ascontiguousarray(xh[i]),
            "xl": np.ascontiguousarray(xl[i]),
            "wpack": _cached_weights,
        }
        for i in range(N_CORES)
    ]
    times = []
    for _ in range(max(1, REPS)):
        res = run_bass_kernel_spmd(
            _cached_nc,
            in_maps,
            core_ids=list(range(N_CORES)),
            trace=TRACE,
            trace_cores=TRACE_CORES,
        )
        if res.exec_time_ns is not None:
            times.append(res.exec_time_ns)
    LAST_ALL_NS = times
    LAST_EXEC_NS = min(times) if times else None
    LAST_RESULTS = res
    return np.stack([r["y"] for r in res.results], axis=0)
